# revision 52
# baseline (speedup 1.0000x reference)
"""BinaryDense kernel for Trainium2: out = sign(x) @ sign(w).

Full shapes: x [8192, 4096] f32, w [4096, 4096] f32 -> out [8192, 4096] f32.
Sharding over 8 NeuronCores: x rows split 4 ways, w columns split 2 ways;
each core computes a [2048, 2048] block.  No collectives.

Core ideas (cost-model-driven):
  - fp8e5 cast-loads (SWDGE): IEEE casts preserve the sign BIT (even on
    underflow to +-0) and only the sign bit matters -> input DMA halves.
  - Bitwise sign on uint16 views: (r & 0x8080) | 0x3C3C == +-1.0 fp8e5 in
    both packed bytes.  Single DVE op per 2 elements, exact.
  - Matmuls: fp8 DoubleRowSwInterleave (0.5 cycles/row).  The k-pair-packed
    transposed x IS the interleaved stationary operand; w cast-loads land
    directly in the plane-separated moving layout.  The mode reads
    stationary columns in reverse order, so the host pre-reverses x rows
    within each 128-row block.
  - PSUM f32 accumulation is exact (+-1 products); int16 out; host widens.

Schedule (slab-pipelined): all input DMAs share one serialized device, so
delivery ORDER is everything.  w arrives as four k-complete 512-column
slabs (each slab: 4 quad-j loads), so each completed slab unlocks a full
m-block sweep on the PE instead of capping pre-stream work at the 2
m-blocks PSUM can j-accumulate.  x chunks 0-1 are PE-transposed early
(w-independent PE work during the stream); chunks 2-7 use XBAR DMA
transposes in the post-stream DMA slack.  wsgn is slab-major so quad
loads land contiguous and sign ops are flat in-place 2-D DVE ops.

Queue map: Pool=cast loads | DVE=signs+evictions | SP=XBAR transposes |
ACT=output DMA issue | PE=matmuls + chunk-0/1 transposes.
"""

import numpy as np

import concourse.mybir as mybir
import concourse.tile as tile
from concourse import bacc
from concourse.bass_utils import run_bass_kernel_spmd
from concourse.masks import make_identity

P = 128
N_CORES = 8
RM, RN = 4, 2
M_FULL, K, N_FULL = 8192, 4096, 4096
M_SH, N_SH = M_FULL // RM, N_FULL // RN   # 2048, 2048
MB = M_SH // P           # 16 m-blocks
JB = K // 256            # 16 k-groups (DoubleRow: 2 planes x 128)
NB = N_SH // 512         # 4 psum-width slabs
XC = MB // 2             # 8 x-chunks of 2 m-blocks
TGRP = 8                 # u16 128-blocks per PE-transpose psum group
QJ = 4                   # j's per w quad-load
NQ = JB // QJ            # 4 quad-loads per slab

F32 = mybir.dt.float32
FP8 = mybir.dt.float8e5
U16 = mybir.dt.uint16
I16 = mybir.dt.int16

AND_MASK = 0x8080
OR_MASK = 0x3C3C
DRSW = mybir.MatmulPerfMode.DoubleRowSwInterleave

_NC_CACHE = None


def build_nc():
    nc = bacc.Bacc("TRN2", target_bir_lowering=False, debug=False,
                   num_devices=N_CORES)
    x = nc.dram_tensor("x", [M_SH, K], F32, kind="ExternalInput").ap()
    w = nc.dram_tensor("w", [K, N_SH], F32, kind="ExternalInput").ap()
    out = nc.dram_tensor("out", [M_SH, N_SH], I16, kind="ExternalOutput").ap()

    with tile.TileContext(nc) as tc:
        with (
            tc.tile_pool(name="const", bufs=1) as const_pool,
            tc.tile_pool(name="xT", bufs=1) as xT_pool,
            tc.tile_pool(name="wbin", bufs=1) as w_pool,
            tc.tile_pool(name="xs", bufs=5) as xs_pool,
            tc.tile_pool(name="obuf", bufs=8) as ob_pool,
            tc.tile_pool(name="psum", bufs=6, space="PSUM") as psum_pool,
            tc.tile_pool(name="psumT", bufs=2, space="PSUM") as psumT_pool,
        ):
            ident = const_pool.tile([P, P], mybir.dt.int16)

            # xT u16[p, mb, j, m] = fp8 pair (k=256j+2p, +1) of row m
            xT = xT_pool.tile([P, MB, JB, P], U16)
            # wsgn[p, s, j, t, n] = sign(w[256j+2p+t, 512s+n]) -- slab-major
            # so quad loads land contiguous and signs are flat 2-D in-place.
            wsgn = w_pool.tile([P, NB, JB, 2, 512], FP8)
            w4d = w.rearrange("(j p t) n -> p j t n", p=P, t=2)
            # x chunk c covers m-blocks 2c, 2c+1: partition p holds rows
            # 256c+p and 256c+128+p
            x3d = x.rearrange("(c two p) k -> p c two k", two=2, p=P)

            xstage = [None] * XC

            def sign_u16(dst, src):
                nc.vector.tensor_scalar(
                    dst, src, AND_MASK, OR_MASK,
                    mybir.AluOpType.bitwise_and, mybir.AluOpType.bitwise_or)

            def load_w_plane(s, t, j0=0, j1=JB):
                # 3-D balanced load: [128, j, 512] one t-plane of slab s
                nc.gpsimd.dma_start(
                    out=wsgn[:, s, j0:j1, t, :],
                    in_=w4d[:, j0:j1, t, 512 * s:512 * (s + 1)])

            def sign_w_half(s, jh):
                # j-half block of slab s is contiguous: flat 2-D in-place sign
                v = wsgn[:, s, 8 * jh:8 * (jh + 1), :, :].bitcast(U16)
                flat = v.rearrange("p a t n -> p (a t n)")
                sign_u16(flat, flat)

            def load_x_raw(c):
                xs = xs_pool.tile([P, 2, K], FP8, tag="xs")
                nc.gpsimd.dma_start(out=xs[:], in_=x3d[:, c])
                xstage[c] = xs

            def load_x_raw_half(c, half):
                if half == 0:
                    xstage[c] = xs_pool.tile([P, 2, K], FP8, tag="xs",
                                             name="xsh")
                nc.gpsimd.dma_start(
                    out=xstage[c][:, half, :], in_=x3d[:, c, half, :])

            def sign_x(c):
                sign_u16(xstage[c][:].bitcast(U16), xstage[c][:].bitcast(U16))

            def pe_transpose_group(c, half, g):
                xu = xstage[c][:].bitcast(mybir.dt.bfloat16)
                mbi = 2 * c + half
                pt = psumT_pool.tile([P, 512], F32, tag="pt", name="pt")
                ptb = pt[:].bitcast(mybir.dt.bfloat16)
                for i in range(TGRP):
                    b = TGRP * g + i
                    nc.tensor.transpose(
                        ptb[:, i * P:(i + 1) * P],
                        xu[:, half, b * P:(b + 1) * P],
                        ident[:].bitcast(mybir.dt.bfloat16))
                sign_u16(xT[:, mbi, TGRP * g:TGRP * (g + 1), :],
                         pt[:].bitcast(U16).rearrange(
                             "p (a b) -> p a b", a=TGRP))

            def pe_transpose_half(c, half):
                # One m-block (16 u16-blocks) of chunk c through the PE in
                # two TGRP groups; sign is fused into the DVE eviction.
                # The PE transpose runs on BF16 *views* of the u16 pair
                # data: transpose mode is pure routing and bit-preserving,
                # and bf16 is a compiler-accepted PE dtype while u16 is not.
                xu = xstage[c][:].bitcast(mybir.dt.bfloat16)   # [P, 2, 2048]
                mbi = 2 * c + half
                for g in range(2):
                    pt = psumT_pool.tile([P, 512], F32, tag="pt", name="pt")
                    ptb = pt[:].bitcast(mybir.dt.bfloat16)     # [P, 1024]
                    for i in range(TGRP):
                        b = TGRP * g + i
                        nc.tensor.transpose(
                            ptb[:, i * P:(i + 1) * P],
                            xu[:, half, b * P:(b + 1) * P],
                            ident[:].bitcast(mybir.dt.bfloat16))
                    sign_u16(xT[:, mbi, TGRP * g:TGRP * (g + 1), :],
                             pt[:].bitcast(U16).rearrange(
                                 "p (a b) -> p a b", a=TGRP))

            def transpose_x(c):
                nc.sync.dma_start(
                    out=xT[:, 2 * c:2 * c + 2, :, :],
                    in_=xstage[c][:].bitcast(U16), transpose=True)

            def mm(po, mb, j, s, start, stop, off=0, wd=512):
                nc.tensor.matmul(
                    po[:, 0:wd], xT[:, mb, j, :].bitcast(FP8),
                    wsgn[:, s, j, :, off:off + wd],
                    start=start, stop=stop, perf_mode=DRSW)

            def sign_x_half(c, half):
                v = xstage[c][:, half, :].bitcast(U16)
                sign_u16(v, v)

            def transpose_x_half(c, half):
                nc.sync.dma_start(
                    out=xT[:, 2 * c + half, :, :],
                    in_=xstage[c][:, half, :].bitcast(U16), transpose=True)

            # ---- Pool load stream: chunks 0-2 load before the
            # remaining w so the slab phase can sweep SIX m-blocks
            # (6 psum accumulators) per slab -- consumption (10.3us/slab)
            # then outpaces delivery (5.8us/slab) and the PE is the pole
            # from ~9us onward.
            load_x_raw_half(0, 0)
            make_identity(nc, ident)
            load_w_plane(0, 0, 0, 8)
            load_w_plane(0, 1, 0, 8)
            load_x_raw_half(0, 1)
            load_x_raw(1)
            load_x_raw_half(2, 0)
            load_x_raw_half(2, 1)
            load_w_plane(0, 0, 8, JB)
            load_w_plane(0, 1, 8, JB)
            for s in range(1, NB):
                for jh in (0, 1):
                    load_w_plane(s, 0, 8 * jh, 8 * (jh + 1))
                    load_w_plane(s, 1, 8 * jh, 8 * (jh + 1))
            load_x_raw_half(3, 0)
            load_x_raw_half(3, 1)

            NSL = 6           # slab-phase m-blocks (psum accumulators)
            ob05 = [ob_pool.tile([P, N_SH], I16, tag="ob", name=f"ob{m}")
                    for m in range(NSL)]
            po6 = [psum_pool.tile([P, 512], F32, tag="po", name="po6")
                   for _ in range(NSL)]

            v000 = wsgn[:, 0, 0:8, 0, :].bitcast(U16)
            sign_u16(v000, v000)
            pe_transpose_group(0, 0, 0)
            pe_transpose_group(0, 0, 1)
            v001 = wsgn[:, 0, 0:8, 1, :].bitcast(U16)
            sign_u16(v001, v001)
            for j in range(8):
                mm(po6[0], 0, j, 0, start=(j == 0), stop=False)
            pe_transpose_half(0, 1)
            for j in range(8):
                mm(po6[1], 1, j, 0, start=(j == 0), stop=False)
            pe_transpose_half(1, 0)
            pe_transpose_half(1, 1)
            for mb in (2, 3):
                for j in range(8):
                    mm(po6[mb], mb, j, 0, start=(j == 0), stop=False)
            pe_transpose_half(2, 0)
            pe_transpose_half(2, 1)
            for mb in (4, 5):
                for j in range(8):
                    mm(po6[mb], mb, j, 0, start=(j == 0), stop=False)
            sign_w_half(0, 1)
            for s in range(1, NB):
                sign_w_half(s, 0)
                sign_w_half(s, 1)
            sign_x_half(3, 0)
            sign_x_half(3, 1)
            transpose_x_half(3, 0)
            transpose_x_half(3, 1)

            # slab sweeps: each m-block's chunk evicts right after its mms
            # so the psum bank recycles before the next slab needs it
            for s in range(NB):
                nsl = slice(512 * s, 512 * (s + 1))
                if s > 0:
                    po6 = [psum_pool.tile([P, 512], F32, tag="po",
                                          name="po6")
                           for _ in range(NSL)]
                    for mb in range(NSL):
                        for j in range(8):
                            mm(po6[mb], mb, j, s,
                               start=(j == 0), stop=False)
                for mb in range(NSL):
                    for j in range(8, JB):
                        mm(po6[mb], mb, j, s,
                           start=False, stop=(j == JB - 1))
                    nc.scalar.copy(out=ob05[mb][:, nsl], in_=po6[mb][:])

            # ---- steady: mb6..15; ungated half-chunk chains (the slab
            # phase leaves them ~15us of slack); slab-row stores issue
            # from SP interleaved with the XBARs
            for mb in range(NSL, MB):
                # pin each steady iteration at its achievable start so the
                # scheduler's static plan (and the tick-waits it inserts)
                # matches the runtime order: without this it believes the
                # x chains complete early and chains the first steady
                # eviction behind ALL of them.
                t0 = 56.0 + (mb - NSL) * 6.85
                # x chains pinned ahead of the compute pin so their
                # load->sign->XBAR latency lands before the PE needs them
                tc.tile_set_cur_wait(max(t0 - 6.0, 0.0) / 1000.0)
                nmb = mb + 2
                if nmb < MB:
                    c, half = divmod(nmb, 2)
                    load_x_raw_half(c, half)
                    sign_x_half(c, half)
                    transpose_x_half(c, half)
                tc.tile_set_cur_wait(t0 / 1000.0)
                if mb - NSL < NSL:
                    m = mb - NSL
                    nc.sync.dma_start(out=out[m * P:(m + 1) * P, :],
                                      in_=ob05[m][:])
                last = (mb == MB - 1)
                ob = ob_pool.tile([P, N_SH], I16, tag="ob")
                # the last m-block tapers its final chunks so the exposed
                # end-of-program evict+DMA chain is short
                widths = [(0, 512), (1, 512), (2, 512), (3, 256), (3, 256)] \
                    if last else [(s, 512) for s in range(NB)]
                off_in_s = 0
                prev_s = 0
                for s, wd in widths:
                    if s != prev_s:
                        off_in_s = 0
                        prev_s = s
                    po = psum_pool.tile([P, 512], F32, tag="po", name="po")
                    for j in range(JB):
                        mm(po, mb, j, s, start=(j == 0), stop=(j == JB - 1),
                           off=off_in_s, wd=wd)
                    nsl = slice(512 * s + off_in_s, 512 * s + off_in_s + wd)
                    nc.scalar.copy(out=ob[:, nsl], in_=po[:, 0:wd])
                    if last:
                        # overlap the two final stores on different queues
                        eng = nc.scalar if wd == 448 else nc.sync
                        eng.dma_start(
                            out=out[mb * P:(mb + 1) * P, nsl], in_=ob[:, nsl])
                    off_in_s += wd
                if not last:
                    nc.scalar.dma_start(
                        out=out[mb * P:(mb + 1) * P, :], in_=ob[:])

    nc.compile()
    return nc


def get_nc():
    global _NC_CACHE
    if _NC_CACHE is None:
        _NC_CACHE = build_nc()
    return _NC_CACHE


def kernel(x: np.ndarray, w: np.ndarray) -> np.ndarray:
    x = np.asarray(x, dtype=np.float32)
    w = np.asarray(w, dtype=np.float32)
    assert x.shape == (M_FULL, K) and w.shape == (K, N_FULL)

    nc = get_nc()
    in_maps = []
    for c in range(N_CORES):
        mi, ni = divmod(c, RN)
        # SwInterleave reads stationary columns in reverse order: pre-reverse
        # x rows within each 128-row block so output rows land in order.
        xs = x[mi * M_SH:(mi + 1) * M_SH, :]
        xs = xs.reshape(MB, P, K)[:, ::-1, :].reshape(M_SH, K)
        in_maps.append({
            "x": np.ascontiguousarray(xs),
            "w": np.ascontiguousarray(w[:, ni * N_SH:(ni + 1) * N_SH]),
        })
    res = run_bass_kernel_spmd(nc, in_maps, list(range(N_CORES)))

    out = np.empty((M_FULL, N_FULL), dtype=np.float32)
    for c in range(N_CORES):
        mi, ni = divmod(c, RN)
        out[mi * M_SH:(mi + 1) * M_SH, ni * N_SH:(ni + 1) * N_SH] = \
            res.results[c]["out"].astype(np.float32)
    return out
